# revision 46
# baseline (speedup 1.0000x reference)
"""Trainium2 Bass kernel for nn_EntityRepr (segment_reduce).

Computation (per example):
  gathered[e,m,s,:] = token_reprs[pos[e,m,s], :]
  mentions[e,m,:]   = mean_s gathered
  entity[e,:]       = mean_m mentions
  mask              = ones

Strategy: data-parallel over batch (4 examples per core, 8 cores).
Per example, the gather+mean runs as matmuls on the PE with one-hot
count matrices as the stationary operand:
  A_k[p, em] = #{s : pos[em,s] == 128k+p}     (dense encoding of pos_idx,
                                               prepared host-side)
  mentions   = (1/S) * A^T @ t                (PE, fp16, K=L=512)
  entity     = Gel^T @ mentions               (PE, float32r, from the f32
                                               mention tiles in SBUF)
The entity block for example b is emitted one mention-block late so its
ACT-copy dependency is long satisfied when the in-order PE reaches it;
the last example's entity instead contracts directly against the tokens
with Aent = sum_m A (host-prepared), so the kernel tail never waits on
a PSUM->SBUF copy.

DMA queues: inputs + entity outputs ride the Sync HWDGE ring, mention
outputs the Scalar HWDGE ring (separate FIFOs).
"""

import os
import numpy as np

import concourse.bass as bass
import concourse.bacc as bacc
import concourse.mybir as mybir
from concourse.bass_utils import run_bass_kernel_spmd
from concourse.tile import TileContext

N_CORES = 8
B, L, H = 32, 512, 768
E, M, S = 32, 8, 4
BS = B // N_CORES          # examples per core
EM = E * M                 # 256 mentions / example
EMS = EM * S               # 1024 gathered rows / example
KT = L // 128              # 4 contraction tiles
HH = H // 2                # 384, matmul free-dim tile

F16 = mybir.dt.float16
F32 = mybir.dt.float32
F32R = mybir.dt.float32r
ALU = mybir.AluOpType
ACTF = mybir.ActivationFunctionType

# set by test harness to collect a profile
TRACE = bool(int(os.environ.get("KERNEL_TRACE", "0")))
LAST_RESULTS = None


def build_nc():
    nc = bacc.Bacc(None, target_bir_lowering=False)
    t16d = nc.declare_dram_parameter("t16", [BS, L, H], F16, isOutput=False)
    # a_in[b, p, k, em] = #{s : pos[b, em, s] == 128k+p}
    a_in = nc.declare_dram_parameter("a_in", [BS, 128, KT, EM], F16, isOutput=False)
    # aent3[p, k, e] = sum_m a_in[BS-1, p, k, e*M+m]
    aent3 = nc.declare_dram_parameter("aent3", [128, KT, E], F16, isOutput=False)
    gel = nc.declare_dram_parameter("gel", [128, 16], F32R, isOutput=False)
    mentions = nc.declare_dram_parameter("mentions", [BS, EM, H], F32, isOutput=True)
    entity = nc.declare_dram_parameter("entity", [BS, E, H], F32, isOutput=True)

    with TileContext(nc) as tc:
        with (
            tc.tile_pool(name="consts", bufs=1) as consts,
            tc.tile_pool(name="tin", bufs=4) as tin,
            tc.tile_pool(name="ain", bufs=4) as ain_pool,
            tc.tile_pool(name="mf32p", bufs=8) as mf32p,
            tc.tile_pool(name="entp", bufs=4) as entp,
            tc.tile_pool(name="psq", bufs=8, space="PSUM") as psq,
        ):
            # input DMAs, earliest-needed first; one-hot counts ride the
            # Scalar ring, tokens the Sync ring, so the two first-example
            # dependencies land in parallel. b0's tokens arrive per-k so the
            # first matmul can start as soon as k=0 is resident.
            ats, t16s = [], []
            at0 = ain_pool.tile([128, KT, EM], F16, tag="at")
            nc.scalar.dma_start(at0[:], a_in[0])
            ats.append(at0)
            # b0's tokens split per-k across both rings so the first mention
            # block can start as soon as k=0 lands
            # b0's k0/k1 on sync, k2/k3 on scalar: both rings are otherwise
            # idle during the head, so all four k-tiles land ~in parallel.
            # The b1..b3 conveyor follows immediately on sync, ahead of any
            # output traffic; the scalar ring stays clear once ACT copies
            # begin.
            # four separate tiles so the two rings never co-write one tile
            # (Tile serializes cross-queue writers of a shared tile)
            t16_0 = []
            for k in range(KT):
                t0k = tin.tile([128, H], F16, tag="t0k", name=f"t0k_{k}")
                eng = nc.sync if k < 2 else nc.scalar
                eng.dma_start(t0k[:], t16d[0, 128 * k : 128 * (k + 1), :])
                t16_0.append(t0k)
            t16s.append(t16_0)
            for b in range(1, BS):
                t16 = tin.tile([128, KT, H], F16, tag="t16")
                nc.sync.dma_start(
                    t16[:], t16d[b].rearrange("(k p) h -> p k h", p=128)
                )
                t16s.append(t16)
            for b in range(1, BS):
                at = ain_pool.tile([128, KT, EM], F16, tag="at")
                nc.scalar.dma_start(at[:], a_in[b])
                ats.append(at)
            gel_sb = consts.tile([128, 16], F32R)
            nc.scalar.dma_start(gel_sb[:], gel[:])
            aent3_sb = consts.tile([128, KT, E], F16)
            nc.scalar.dma_start(aent3_sb[:], aent3[:])

            # PE warm-up: HAM releases the clock throttle after ~3.4us of
            # sustained matmul activity; burn that in during the DMA head so
            # the real matmuls run at 2.4 GHz from the start.
            scratch = consts.tile([128, 512], F16)
            nc.gpsimd.memset(scratch[:], 0.0)
            wm = psq.tile([128, 512], F32, tag="ps")
            for _ in range(12):
                nc.tensor.matmul(
                    wm[:], lhsT=scratch[:, 0:128], rhs=scratch[:],
                    start=True, stop=True,
                )

            def mention_block(b):
                # mentions = (1/S) * A^T @ t   (two 128-row chunks of em)
                mfs = []
                for e2 in range(2):
                    pmh = [
                        psq.tile([128, 512], F32, tag="ps", name=f"pm_{b}_{e2}_{h}")
                        for h in range(2)
                    ]
                    for k in range(KT):
                        for h2 in range(2):
                            nc.tensor.matmul(
                                pmh[h2][:, 0:HH],
                                lhsT=ats[b][:, k, 128 * e2 : 128 * (e2 + 1)],
                                rhs=(
                                    t16s[b][k][:, HH * h2 : HH * (h2 + 1)]
                                    if b == 0
                                    else t16s[b][:, k, HH * h2 : HH * (h2 + 1)]
                                ),
                                start=(k == 0),
                                stop=(k == KT - 1),
                            )
                    mf = mf32p.tile([128, 2, HH], F32R, tag="mf")
                    for h2 in range(2):
                        nc.scalar.activation(
                            mf[:, h2, :], pmh[h2][:, 0:HH], ACTF.Copy, 0.0, 1.0 / S
                        )
                    nc.sync.dma_start(
                        mentions[b, 128 * e2 : 128 * (e2 + 1), :].rearrange(
                            "p (c h) -> p c h", c=2
                        ),
                        mf[:].bitcast(F32),
                    )
                    mfs.append(mf)
                return mfs

            def entity_block(b, mfs):
                # entity = Gel^T @ mentions  (float32r full-rate matmul)
                for e2 in range(2):
                    peh = [
                        psq.tile([16, 512], F32, tag="ps", name=f"pe_{b}_{e2}_{h}")
                        for h in range(2)
                    ]
                    ent = entp.tile([16, 2, HH], F32, tag="ent")
                    for h2 in range(2):
                        nc.tensor.matmul(
                            peh[h2][:, 0:HH],
                            lhsT=gel_sb[:],
                            rhs=mfs[e2][:, h2, :],
                            start=True,
                            stop=True,
                        )
                        nc.vector.tensor_scalar_mul(
                            ent[:, h2, :], peh[h2][:, 0:HH], 1.0
                        )
                    nc.sync.dma_start(
                        entity[b, 16 * e2 : 16 * (e2 + 1), :].rearrange(
                            "p (g h) -> p g h", g=2
                        ),
                        ent[:],
                    )

            def entity_block_direct(b):
                # entity = (1/(S*M)) * Aent^T @ t — no dependency on the
                # mention copies, so the kernel tail isn't gated on ACT
                peh3 = [
                    psq.tile([32, 512], F32, tag="ps", name=f"pe3_{h}")
                    for h in range(2)
                ]
                ent = entp.tile([32, 2, HH], F32, tag="ent")
                for h2 in range(2):
                    for k in range(KT):
                        nc.tensor.matmul(
                            peh3[h2][:, 0:HH],
                            lhsT=aent3_sb[:, k, :],
                            rhs=t16s[b][:, k, HH * h2 : HH * (h2 + 1)],
                            start=(k == 0),
                            stop=(k == KT - 1),
                        )
                    nc.vector.tensor_scalar_mul(
                        ent[:, h2, :], peh3[h2][:, 0:HH], 1.0 / (S * M)
                    )
                nc.sync.dma_start(
                    entity[b].rearrange("e (g h) -> e g h", g=2),
                    ent[:],
                )

            # entity block for example b runs one mention-block late so its
            # mf dependency (an ACT copy) is long satisfied when PE gets
            # there; the direct (token-contracting) block for the last
            # example runs even earlier so only short copies trail the
            # final mention block
            all_mfs = []
            for b in range(BS):
                if b == BS - 1:
                    entity_block_direct(BS - 1)
                all_mfs.append(mention_block(b))
                if b >= 1:
                    entity_block(b - 1, all_mfs[b - 1])
    nc.compile()
    return nc


_NC = None


def _get_nc():
    global _NC
    if _NC is None:
        _NC = build_nc()
    return _NC


def make_host_inputs(token_reprs, pos_idx):
    """Split + preprocess full inputs into per-core in_maps."""
    token_reprs = np.asarray(token_reprs)
    pos = np.asarray(pos_idx).astype(np.int64)
    assert token_reprs.shape == (B, L, H)
    assert pos.shape == (B, E, M, S)
    t16 = np.ascontiguousarray(token_reprs.astype(np.float16))

    # dense one-hot count encoding of pos_idx: A[b, l, em] = #{s: pos==l}
    pos_flat = pos.reshape(B, EM, S)
    b_idx = np.arange(B)[:, None, None]
    em_idx = np.arange(EM)[None, :, None]
    lin = (b_idx * L + pos_flat) * EM + em_idx
    counts = np.bincount(lin.ravel(), minlength=B * L * EM).reshape(B, L, EM)
    # a_in[b, p, k, em]
    a_np = np.ascontiguousarray(
        counts.reshape(B, KT, 128, EM).transpose(0, 2, 1, 3).astype(np.float16)
    )
    # aent[b, p, k, e] = sum_m counts
    aent_np = np.ascontiguousarray(
        counts.reshape(B, KT, 128, E, M).sum(axis=4).transpose(0, 2, 1, 3)
        .astype(np.float16)
    )
    gel = np.zeros((128, 16), np.float32)
    gel[np.arange(128), np.arange(128) // M] = 1.0 / M

    in_maps = []
    for c in range(N_CORES):
        sl = slice(c * BS, (c + 1) * BS)
        in_maps.append(
            {
                "t16": t16[sl],
                "a_in": a_np[sl],
                "aent3": aent_np[c * BS + BS - 1],
                "gel": gel,
            }
        )
    return in_maps


def kernel(token_reprs, pos_idx):
    global LAST_RESULTS
    in_maps = make_host_inputs(token_reprs, pos_idx)
    nc = _get_nc()
    res = run_bass_kernel_spmd(nc, in_maps, list(range(N_CORES)), trace=TRACE)
    LAST_RESULTS = res
    mentions = np.concatenate([r["mentions"] for r in res.results], axis=0).reshape(
        B, E, M, H
    )
    entity = np.concatenate([r["entity"] for r in res.results], axis=0)
    mask = np.ones((B, E, M), np.float32)
    return entity, mentions, mask


# revision 47
# speedup vs baseline: 1.0275x; 1.0275x over previous
"""Trainium2 Bass kernel for nn_EntityRepr (segment_reduce).

Computation (per example):
  gathered[e,m,s,:] = token_reprs[pos[e,m,s], :]
  mentions[e,m,:]   = mean_s gathered
  entity[e,:]       = mean_m mentions
  mask              = ones

Strategy: data-parallel over batch (4 examples per core, 8 cores).
Per example, the gather+mean runs as matmuls on the PE with one-hot
count matrices as the stationary operand:
  A_k[p, em] = #{s : pos[em,s] == 128k+p}     (dense encoding of pos_idx,
                                               prepared host-side)
  mentions   = (1/S) * A^T @ t                (PE, fp16, K=L=512)
  entity     = Gel^T @ mentions               (PE, float32r, from the f32
                                               mention tiles in SBUF)
The entity block for example b is emitted one mention-block late so its
ACT-copy dependency is long satisfied when the in-order PE reaches it;
the last example's entity instead contracts directly against the tokens
with Aent = sum_m A (host-prepared), so the kernel tail never waits on
a PSUM->SBUF copy.

DMA queues: inputs + entity outputs ride the Sync HWDGE ring, mention
outputs the Scalar HWDGE ring (separate FIFOs).
"""

import os
import numpy as np

import concourse.bass as bass
import concourse.bacc as bacc
import concourse.mybir as mybir
from concourse.bass_utils import run_bass_kernel_spmd
from concourse.tile import TileContext

N_CORES = 8
B, L, H = 32, 512, 768
E, M, S = 32, 8, 4
BS = B // N_CORES          # examples per core
EM = E * M                 # 256 mentions / example
EMS = EM * S               # 1024 gathered rows / example
KT = L // 128              # 4 contraction tiles
HH = H // 2                # 384, matmul free-dim tile

F16 = mybir.dt.float16
F32 = mybir.dt.float32
F32R = mybir.dt.float32r
ALU = mybir.AluOpType
ACTF = mybir.ActivationFunctionType

# set by test harness to collect a profile
TRACE = bool(int(os.environ.get("KERNEL_TRACE", "0")))
LAST_RESULTS = None


def build_nc():
    nc = bacc.Bacc(None, target_bir_lowering=False)
    t16d = nc.declare_dram_parameter("t16", [BS, L, H], F16, isOutput=False)
    # a_in[b, p, k, em] = #{s : pos[b, em, s] == 128k+p}
    a_in = nc.declare_dram_parameter("a_in", [BS, 128, KT, EM], F16, isOutput=False)
    # aent3[p, k, e] = sum_m a_in[BS-1, p, k, e*M+m]
    aent3 = nc.declare_dram_parameter("aent3", [128, KT, E], F16, isOutput=False)
    gel = nc.declare_dram_parameter("gel", [128, 16], F32R, isOutput=False)
    mentions = nc.declare_dram_parameter("mentions", [BS, EM, H], F32, isOutput=True)
    entity = nc.declare_dram_parameter("entity", [BS, E, H], F32, isOutput=True)

    with TileContext(nc) as tc:
        with (
            tc.tile_pool(name="consts", bufs=1) as consts,
            tc.tile_pool(name="tin", bufs=4) as tin,
            tc.tile_pool(name="ain", bufs=4) as ain_pool,
            tc.tile_pool(name="mf32p", bufs=8) as mf32p,
            tc.tile_pool(name="entp", bufs=4) as entp,
            tc.tile_pool(name="psq", bufs=8, space="PSUM") as psq,
        ):
            # input DMAs, earliest-needed first; one-hot counts ride the
            # Scalar ring, tokens the Sync ring, so the two first-example
            # dependencies land in parallel. b0's tokens arrive per-k so the
            # first matmul can start as soon as k=0 is resident.
            ats, t16s = [], []
            at0 = ain_pool.tile([128, KT, EM], F16, tag="at")
            nc.scalar.dma_start(at0[:], a_in[0])
            ats.append(at0)
            # b0's tokens split per-k across both rings so the first mention
            # block can start as soon as k=0 lands
            # b0's k0/k1 on sync, k2/k3 on scalar: both rings are otherwise
            # idle during the head, so all four k-tiles land ~in parallel.
            # The b1..b3 conveyor follows immediately on sync, ahead of any
            # output traffic; the scalar ring stays clear once ACT copies
            # begin.
            # four separate tiles so the two rings never co-write one tile
            # (Tile serializes cross-queue writers of a shared tile)
            t16_0 = []
            for k in range(KT):
                t0k = tin.tile([128, H], F16, tag="t0k", name=f"t0k_{k}")
                eng = nc.sync if k < 2 else nc.scalar
                eng.dma_start(t0k[:], t16d[0, 128 * k : 128 * (k + 1), :])
                t16_0.append(t0k)
            t16s.append(t16_0)
            # b1 also arrives per-k (single ring, one tile) so its first
            # k-tiles are resident before b0's block finishes
            t16_1 = tin.tile([128, KT, H], F16, tag="t16")
            for k in range(KT):
                nc.sync.dma_start(
                    t16_1[:, k, :], t16d[1, 128 * k : 128 * (k + 1), :]
                )
            t16s.append(t16_1)
            for b in range(2, BS):
                t16 = tin.tile([128, KT, H], F16, tag="t16")
                nc.sync.dma_start(
                    t16[:], t16d[b].rearrange("(k p) h -> p k h", p=128)
                )
                t16s.append(t16)
            for b in range(1, BS):
                at = ain_pool.tile([128, KT, EM], F16, tag="at")
                nc.scalar.dma_start(at[:], a_in[b])
                ats.append(at)
            gel_sb = consts.tile([128, 16], F32R)
            nc.scalar.dma_start(gel_sb[:], gel[:])
            aent3_sb = consts.tile([128, KT, E], F16)
            nc.scalar.dma_start(aent3_sb[:], aent3[:])

            # PE warm-up: HAM releases the clock throttle after ~3.4us of
            # sustained matmul activity; burn that in during the DMA head so
            # the real matmuls run at 2.4 GHz from the start.
            scratch = consts.tile([128, 512], F16)
            nc.gpsimd.memset(scratch[:], 0.0)
            wm = psq.tile([128, 512], F32, tag="ps")
            for _ in range(12):
                nc.tensor.matmul(
                    wm[:], lhsT=scratch[:, 0:128], rhs=scratch[:],
                    start=True, stop=True,
                )

            def mention_block(b):
                # mentions = (1/S) * A^T @ t   (two 128-row chunks of em)
                mfs = []
                for e2 in range(2):
                    pmh = [
                        psq.tile([128, 512], F32, tag="ps", name=f"pm_{b}_{e2}_{h}")
                        for h in range(2)
                    ]
                    for k in range(KT):
                        for h2 in range(2):
                            nc.tensor.matmul(
                                pmh[h2][:, 0:HH],
                                lhsT=ats[b][:, k, 128 * e2 : 128 * (e2 + 1)],
                                rhs=(
                                    t16s[b][k][:, HH * h2 : HH * (h2 + 1)]
                                    if b == 0
                                    else t16s[b][:, k, HH * h2 : HH * (h2 + 1)]
                                ),
                                start=(k == 0),
                                stop=(k == KT - 1),
                            )
                    mf = mf32p.tile([128, 2, HH], F32R, tag="mf")
                    for h2 in range(2):
                        nc.scalar.activation(
                            mf[:, h2, :], pmh[h2][:, 0:HH], ACTF.Copy, 0.0, 1.0 / S
                        )
                    nc.sync.dma_start(
                        mentions[b, 128 * e2 : 128 * (e2 + 1), :].rearrange(
                            "p (c h) -> p c h", c=2
                        ),
                        mf[:].bitcast(F32),
                    )
                    mfs.append(mf)
                return mfs

            def entity_block(b, mfs):
                # entity = Gel^T @ mentions  (float32r full-rate matmul)
                for e2 in range(2):
                    peh = [
                        psq.tile([16, 512], F32, tag="ps", name=f"pe_{b}_{e2}_{h}")
                        for h in range(2)
                    ]
                    ent = entp.tile([16, 2, HH], F32, tag="ent")
                    for h2 in range(2):
                        nc.tensor.matmul(
                            peh[h2][:, 0:HH],
                            lhsT=gel_sb[:],
                            rhs=mfs[e2][:, h2, :],
                            start=True,
                            stop=True,
                        )
                        nc.vector.tensor_scalar_mul(
                            ent[:, h2, :], peh[h2][:, 0:HH], 1.0
                        )
                    nc.sync.dma_start(
                        entity[b, 16 * e2 : 16 * (e2 + 1), :].rearrange(
                            "p (g h) -> p g h", g=2
                        ),
                        ent[:],
                    )

            def entity_block_direct(b):
                # entity = (1/(S*M)) * Aent^T @ t — no dependency on the
                # mention copies, so the kernel tail isn't gated on ACT
                peh3 = [
                    psq.tile([32, 512], F32, tag="ps", name=f"pe3_{h}")
                    for h in range(2)
                ]
                ent = entp.tile([32, 2, HH], F32, tag="ent")
                for h2 in range(2):
                    for k in range(KT):
                        nc.tensor.matmul(
                            peh3[h2][:, 0:HH],
                            lhsT=aent3_sb[:, k, :],
                            rhs=t16s[b][:, k, HH * h2 : HH * (h2 + 1)],
                            start=(k == 0),
                            stop=(k == KT - 1),
                        )
                    nc.vector.tensor_scalar_mul(
                        ent[:, h2, :], peh3[h2][:, 0:HH], 1.0 / (S * M)
                    )
                nc.sync.dma_start(
                    entity[b].rearrange("e (g h) -> e g h", g=2),
                    ent[:],
                )

            # entity block for example b runs one mention-block late so its
            # mf dependency (an ACT copy) is long satisfied when PE gets
            # there; the direct (token-contracting) block for the last
            # example runs even earlier so only short copies trail the
            # final mention block
            all_mfs = []
            for b in range(BS):
                if b == BS - 1:
                    entity_block_direct(BS - 1)
                all_mfs.append(mention_block(b))
                if b >= 1:
                    entity_block(b - 1, all_mfs[b - 1])
    nc.compile()
    return nc


_NC = None


def _get_nc():
    global _NC
    if _NC is None:
        _NC = build_nc()
    return _NC


def make_host_inputs(token_reprs, pos_idx):
    """Split + preprocess full inputs into per-core in_maps."""
    token_reprs = np.asarray(token_reprs)
    pos = np.asarray(pos_idx).astype(np.int64)
    assert token_reprs.shape == (B, L, H)
    assert pos.shape == (B, E, M, S)
    t16 = np.ascontiguousarray(token_reprs.astype(np.float16))

    # dense one-hot count encoding of pos_idx: A[b, l, em] = #{s: pos==l}
    pos_flat = pos.reshape(B, EM, S)
    b_idx = np.arange(B)[:, None, None]
    em_idx = np.arange(EM)[None, :, None]
    lin = (b_idx * L + pos_flat) * EM + em_idx
    counts = np.bincount(lin.ravel(), minlength=B * L * EM).reshape(B, L, EM)
    # a_in[b, p, k, em]
    a_np = np.ascontiguousarray(
        counts.reshape(B, KT, 128, EM).transpose(0, 2, 1, 3).astype(np.float16)
    )
    # aent[b, p, k, e] = sum_m counts
    aent_np = np.ascontiguousarray(
        counts.reshape(B, KT, 128, E, M).sum(axis=4).transpose(0, 2, 1, 3)
        .astype(np.float16)
    )
    gel = np.zeros((128, 16), np.float32)
    gel[np.arange(128), np.arange(128) // M] = 1.0 / M

    in_maps = []
    for c in range(N_CORES):
        sl = slice(c * BS, (c + 1) * BS)
        in_maps.append(
            {
                "t16": t16[sl],
                "a_in": a_np[sl],
                "aent3": aent_np[c * BS + BS - 1],
                "gel": gel,
            }
        )
    return in_maps


def kernel(token_reprs, pos_idx):
    global LAST_RESULTS
    in_maps = make_host_inputs(token_reprs, pos_idx)
    nc = _get_nc()
    res = run_bass_kernel_spmd(nc, in_maps, list(range(N_CORES)), trace=TRACE)
    LAST_RESULTS = res
    mentions = np.concatenate([r["mentions"] for r in res.results], axis=0).reshape(
        B, E, M, H
    )
    entity = np.concatenate([r["entity"] for r in res.results], axis=0)
    mask = np.ones((B, E, M), np.float32)
    return entity, mentions, mask


# revision 48
# speedup vs baseline: 1.0381x; 1.0103x over previous
"""Trainium2 Bass kernel for nn_EntityRepr (segment_reduce).

Computation (per example):
  gathered[e,m,s,:] = token_reprs[pos[e,m,s], :]
  mentions[e,m,:]   = mean_s gathered
  entity[e,:]       = mean_m mentions
  mask              = ones

Strategy: data-parallel over batch (4 examples per core, 8 cores).
Per example, the gather+mean runs as matmuls on the PE with one-hot
count matrices as the stationary operand:
  A_k[p, em] = #{s : pos[em,s] == 128k+p}     (dense encoding of pos_idx,
                                               prepared host-side)
  mentions   = (1/S) * A^T @ t                (PE, fp16, K=L=512)
  entity     = Gel^T @ mentions               (PE, float32r, from the f32
                                               mention tiles in SBUF)
The entity block for example b is emitted one mention-block late so its
ACT-copy dependency is long satisfied when the in-order PE reaches it;
the last example's entity instead contracts directly against the tokens
with Aent = sum_m A (host-prepared), so the kernel tail never waits on
a PSUM->SBUF copy.

DMA queues: inputs + entity outputs ride the Sync HWDGE ring, mention
outputs the Scalar HWDGE ring (separate FIFOs).
"""

import os
import numpy as np

import concourse.bass as bass
import concourse.bacc as bacc
import concourse.mybir as mybir
from concourse.bass_utils import run_bass_kernel_spmd
from concourse.tile import TileContext

N_CORES = 8
B, L, H = 32, 512, 768
E, M, S = 32, 8, 4
BS = B // N_CORES          # examples per core
EM = E * M                 # 256 mentions / example
EMS = EM * S               # 1024 gathered rows / example
KT = L // 128              # 4 contraction tiles
HH = H // 2                # 384, matmul free-dim tile

F16 = mybir.dt.float16
F32 = mybir.dt.float32
F32R = mybir.dt.float32r
ALU = mybir.AluOpType
ACTF = mybir.ActivationFunctionType

# set by test harness to collect a profile
TRACE = bool(int(os.environ.get("KERNEL_TRACE", "0")))
LAST_RESULTS = None


def build_nc():
    nc = bacc.Bacc(None, target_bir_lowering=False)
    t16d = nc.declare_dram_parameter("t16", [BS, L, H], F16, isOutput=False)
    # a_in[b, p, k, em] = #{s : pos[b, em, s] == 128k+p}
    a_in = nc.declare_dram_parameter("a_in", [BS, 128, KT, EM], F16, isOutput=False)
    # aent3[p, k, e] = sum_m a_in[BS-1, p, k, e*M+m]
    aent3 = nc.declare_dram_parameter("aent3", [128, KT, E], F16, isOutput=False)
    gel = nc.declare_dram_parameter("gel", [128, 16], F32R, isOutput=False)
    mentions = nc.declare_dram_parameter("mentions", [BS, EM, H], F32, isOutput=True)
    entity = nc.declare_dram_parameter("entity", [BS, E, H], F32, isOutput=True)

    with TileContext(nc) as tc:
        with (
            tc.tile_pool(name="consts", bufs=1) as consts,
            tc.tile_pool(name="tin", bufs=4) as tin,
            tc.tile_pool(name="ain", bufs=4) as ain_pool,
            tc.tile_pool(name="mf32p", bufs=8) as mf32p,
            tc.tile_pool(name="entp", bufs=4) as entp,
            tc.tile_pool(name="psq", bufs=8, space="PSUM") as psq,
        ):
            # input DMAs, earliest-needed first; one-hot counts ride the
            # Scalar ring, tokens the Sync ring, so the two first-example
            # dependencies land in parallel. b0's tokens arrive per-k so the
            # first matmul can start as soon as k=0 is resident.
            ats, t16s = [], []
            at0 = ain_pool.tile([128, KT, EM], F16, tag="at")
            nc.scalar.dma_start(at0[:], a_in[0])
            ats.append(at0)
            # b0's tokens split per-k across both rings so the first mention
            # block can start as soon as k=0 lands
            # b0's k0/k1 on sync, k2/k3 on scalar: both rings are otherwise
            # idle during the head, so all four k-tiles land ~in parallel.
            # The b1..b3 conveyor follows immediately on sync, ahead of any
            # output traffic; the scalar ring stays clear once ACT copies
            # begin.
            # four separate tiles so the two rings never co-write one tile
            # (Tile serializes cross-queue writers of a shared tile)
            t16_0 = []
            for k in range(KT):
                t0k = tin.tile([128, H], F16, tag="t0k", name=f"t0k_{k}")
                eng = nc.sync if k < 2 else nc.scalar
                eng.dma_start(t0k[:], t16d[0, 128 * k : 128 * (k + 1), :])
                t16_0.append(t0k)
            t16s.append(t16_0)
            # b1 also arrives per-k (single ring, one tile) so its first
            # k-tiles are resident before b0's block finishes
            t16_1 = tin.tile([128, KT, H], F16, tag="t16")
            for k in range(KT):
                nc.sync.dma_start(
                    t16_1[:, k, :], t16d[1, 128 * k : 128 * (k + 1), :]
                )
            t16s.append(t16_1)
            t16_2 = tin.tile([128, KT, H], F16, tag="t16")
            for k in range(KT):
                nc.sync.dma_start(
                    t16_2[:, k, :], t16d[2, 128 * k : 128 * (k + 1), :]
                )
            t16s.append(t16_2)
            for b in range(3, BS):
                t16 = tin.tile([128, KT, H], F16, tag="t16")
                nc.sync.dma_start(
                    t16[:], t16d[b].rearrange("(k p) h -> p k h", p=128)
                )
                t16s.append(t16)
            for b in range(1, BS):
                at = ain_pool.tile([128, KT, EM], F16, tag="at")
                nc.scalar.dma_start(at[:], a_in[b])
                ats.append(at)
            gel_sb = consts.tile([128, 16], F32R)
            nc.scalar.dma_start(gel_sb[:], gel[:])
            aent3_sb = consts.tile([128, KT, E], F16)
            nc.scalar.dma_start(aent3_sb[:], aent3[:])

            # PE warm-up: HAM releases the clock throttle after ~3.4us of
            # sustained matmul activity; burn that in during the DMA head so
            # the real matmuls run at 2.4 GHz from the start.
            scratch = consts.tile([128, 512], F16)
            nc.gpsimd.memset(scratch[:], 0.0)
            wm = psq.tile([128, 512], F32, tag="ps")
            for _ in range(12):
                nc.tensor.matmul(
                    wm[:], lhsT=scratch[:, 0:128], rhs=scratch[:],
                    start=True, stop=True,
                )

            def mention_block(b):
                # mentions = (1/S) * A^T @ t   (two 128-row chunks of em)
                mfs = []
                for e2 in range(2):
                    pmh = [
                        psq.tile([128, 512], F32, tag="ps", name=f"pm_{b}_{e2}_{h}")
                        for h in range(2)
                    ]
                    for k in range(KT):
                        for h2 in range(2):
                            nc.tensor.matmul(
                                pmh[h2][:, 0:HH],
                                lhsT=ats[b][:, k, 128 * e2 : 128 * (e2 + 1)],
                                rhs=(
                                    t16s[b][k][:, HH * h2 : HH * (h2 + 1)]
                                    if b == 0
                                    else t16s[b][:, k, HH * h2 : HH * (h2 + 1)]
                                ),
                                start=(k == 0),
                                stop=(k == KT - 1),
                            )
                    mf = mf32p.tile([128, 2, HH], F32R, tag="mf")
                    for h2 in range(2):
                        nc.scalar.activation(
                            mf[:, h2, :], pmh[h2][:, 0:HH], ACTF.Copy, 0.0, 1.0 / S
                        )
                    nc.sync.dma_start(
                        mentions[b, 128 * e2 : 128 * (e2 + 1), :].rearrange(
                            "p (c h) -> p c h", c=2
                        ),
                        mf[:].bitcast(F32),
                    )
                    mfs.append(mf)
                return mfs

            def entity_block(b, mfs):
                # entity = Gel^T @ mentions  (float32r full-rate matmul)
                for e2 in range(2):
                    peh = [
                        psq.tile([16, 512], F32, tag="ps", name=f"pe_{b}_{e2}_{h}")
                        for h in range(2)
                    ]
                    ent = entp.tile([16, 2, HH], F32, tag="ent")
                    for h2 in range(2):
                        nc.tensor.matmul(
                            peh[h2][:, 0:HH],
                            lhsT=gel_sb[:],
                            rhs=mfs[e2][:, h2, :],
                            start=True,
                            stop=True,
                        )
                        nc.vector.tensor_scalar_mul(
                            ent[:, h2, :], peh[h2][:, 0:HH], 1.0
                        )
                    nc.sync.dma_start(
                        entity[b, 16 * e2 : 16 * (e2 + 1), :].rearrange(
                            "p (g h) -> p g h", g=2
                        ),
                        ent[:],
                    )

            def entity_block_direct(b):
                # entity = (1/(S*M)) * Aent^T @ t — no dependency on the
                # mention copies, so the kernel tail isn't gated on ACT
                peh3 = [
                    psq.tile([32, 512], F32, tag="ps", name=f"pe3_{h}")
                    for h in range(2)
                ]
                ent = entp.tile([32, 2, HH], F32, tag="ent")
                for h2 in range(2):
                    for k in range(KT):
                        nc.tensor.matmul(
                            peh3[h2][:, 0:HH],
                            lhsT=aent3_sb[:, k, :],
                            rhs=t16s[b][:, k, HH * h2 : HH * (h2 + 1)],
                            start=(k == 0),
                            stop=(k == KT - 1),
                        )
                    nc.vector.tensor_scalar_mul(
                        ent[:, h2, :], peh3[h2][:, 0:HH], 1.0 / (S * M)
                    )
                nc.sync.dma_start(
                    entity[b].rearrange("e (g h) -> e g h", g=2),
                    ent[:],
                )

            # entity block for example b runs one mention-block late so its
            # mf dependency (an ACT copy) is long satisfied when PE gets
            # there; the direct (token-contracting) block for the last
            # example runs even earlier so only short copies trail the
            # final mention block
            all_mfs = []
            for b in range(BS):
                if b == BS - 1:
                    entity_block_direct(BS - 1)
                all_mfs.append(mention_block(b))
                if b >= 1:
                    entity_block(b - 1, all_mfs[b - 1])
    nc.compile()
    return nc


_NC = None


def _get_nc():
    global _NC
    if _NC is None:
        _NC = build_nc()
    return _NC


def make_host_inputs(token_reprs, pos_idx):
    """Split + preprocess full inputs into per-core in_maps."""
    token_reprs = np.asarray(token_reprs)
    pos = np.asarray(pos_idx).astype(np.int64)
    assert token_reprs.shape == (B, L, H)
    assert pos.shape == (B, E, M, S)
    t16 = np.ascontiguousarray(token_reprs.astype(np.float16))

    # dense one-hot count encoding of pos_idx: A[b, l, em] = #{s: pos==l}
    pos_flat = pos.reshape(B, EM, S)
    b_idx = np.arange(B)[:, None, None]
    em_idx = np.arange(EM)[None, :, None]
    lin = (b_idx * L + pos_flat) * EM + em_idx
    counts = np.bincount(lin.ravel(), minlength=B * L * EM).reshape(B, L, EM)
    # a_in[b, p, k, em]
    a_np = np.ascontiguousarray(
        counts.reshape(B, KT, 128, EM).transpose(0, 2, 1, 3).astype(np.float16)
    )
    # aent[b, p, k, e] = sum_m counts
    aent_np = np.ascontiguousarray(
        counts.reshape(B, KT, 128, E, M).sum(axis=4).transpose(0, 2, 1, 3)
        .astype(np.float16)
    )
    gel = np.zeros((128, 16), np.float32)
    gel[np.arange(128), np.arange(128) // M] = 1.0 / M

    in_maps = []
    for c in range(N_CORES):
        sl = slice(c * BS, (c + 1) * BS)
        in_maps.append(
            {
                "t16": t16[sl],
                "a_in": a_np[sl],
                "aent3": aent_np[c * BS + BS - 1],
                "gel": gel,
            }
        )
    return in_maps


def kernel(token_reprs, pos_idx):
    global LAST_RESULTS
    in_maps = make_host_inputs(token_reprs, pos_idx)
    nc = _get_nc()
    res = run_bass_kernel_spmd(nc, in_maps, list(range(N_CORES)), trace=TRACE)
    LAST_RESULTS = res
    mentions = np.concatenate([r["mentions"] for r in res.results], axis=0).reshape(
        B, E, M, H
    )
    entity = np.concatenate([r["entity"] for r in res.results], axis=0)
    mask = np.ones((B, E, M), np.float32)
    return entity, mentions, mask


# revision 49
# speedup vs baseline: 1.0938x; 1.0537x over previous
"""Trainium2 Bass kernel for nn_EntityRepr (segment_reduce).

Computation (per example):
  gathered[e,m,s,:] = token_reprs[pos[e,m,s], :]
  mentions[e,m,:]   = mean_s gathered
  entity[e,:]       = mean_m mentions
  mask              = ones

Strategy: data-parallel over batch (4 examples per core, 8 cores).
Per example, the gather+mean runs as matmuls on the PE with one-hot
count matrices as the stationary operand:
  A_k[p, em] = #{s : pos[em,s] == 128k+p}     (dense encoding of pos_idx,
                                               prepared host-side)
  mentions   = (1/S) * A^T @ t                (PE, fp16, K=L=512)
  entity     = Gel^T @ mentions               (PE, float32r, from the f32
                                               mention tiles in SBUF)
The entity block for example b is emitted one mention-block late so its
ACT-copy dependency is long satisfied when the in-order PE reaches it;
the last example's entity instead contracts directly against the tokens
with Aent = sum_m A (host-prepared), so the kernel tail never waits on
a PSUM->SBUF copy.

DMA queues: inputs + entity outputs ride the Sync HWDGE ring, mention
outputs the Scalar HWDGE ring (separate FIFOs).
"""

import os
import numpy as np

import concourse.bass as bass
import concourse.bacc as bacc
import concourse.mybir as mybir
from concourse.bass_utils import run_bass_kernel_spmd
from concourse.tile import TileContext

N_CORES = 8
B, L, H = 32, 512, 768
E, M, S = 32, 8, 4
BS = B // N_CORES          # examples per core
EM = E * M                 # 256 mentions / example
EMS = EM * S               # 1024 gathered rows / example
KT = L // 128              # 4 contraction tiles
HH = H // 2                # 384, matmul free-dim tile

F16 = mybir.dt.float16
F32 = mybir.dt.float32
F32R = mybir.dt.float32r
ALU = mybir.AluOpType
ACTF = mybir.ActivationFunctionType

# set by test harness to collect a profile
TRACE = bool(int(os.environ.get("KERNEL_TRACE", "0")))
LAST_RESULTS = None


def build_nc():
    nc = bacc.Bacc(None, target_bir_lowering=False)
    t16d = nc.declare_dram_parameter("t16", [BS, L, H], F16, isOutput=False)
    # a_in[b, p, k, em] = #{s : pos[b, em, s] == 128k+p}
    a_in = nc.declare_dram_parameter("a_in", [BS, 128, KT, EM], F16, isOutput=False)
    # aent3[p, k, e] = sum_m a_in[BS-1, p, k, e*M+m]
    aent3 = nc.declare_dram_parameter("aent3", [128, KT, E], F16, isOutput=False)
    gel = nc.declare_dram_parameter("gel", [128, 16], F32R, isOutput=False)
    mentions = nc.declare_dram_parameter("mentions", [BS, EM, H], F32, isOutput=True)
    entity = nc.declare_dram_parameter("entity", [BS, E, H], F32, isOutput=True)

    with TileContext(nc) as tc:
        with (
            tc.tile_pool(name="consts", bufs=1) as consts,
            tc.tile_pool(name="tin", bufs=4) as tin,
            tc.tile_pool(name="ain", bufs=4) as ain_pool,
            tc.tile_pool(name="mf32p", bufs=8) as mf32p,
            tc.tile_pool(name="entp", bufs=4) as entp,
            tc.tile_pool(name="psq", bufs=8, space="PSUM") as psq,
        ):
            # input DMAs, earliest-needed first; one-hot counts ride the
            # Scalar ring, tokens the Sync ring, so the two first-example
            # dependencies land in parallel. b0's tokens arrive per-k so the
            # first matmul can start as soon as k=0 is resident.
            ats, t16s = [], []
            at0 = ain_pool.tile([128, KT, EM], F16, tag="at")
            nc.scalar.dma_start(at0[:], a_in[0])
            ats.append(at0)
            # b0's tokens split per-k across both rings so the first mention
            # block can start as soon as k=0 lands
            # b0's k0/k1 on sync, k2/k3 on scalar: both rings are otherwise
            # idle during the head, so all four k-tiles land ~in parallel.
            # The b1..b3 conveyor follows immediately on sync, ahead of any
            # output traffic; the scalar ring stays clear once ACT copies
            # begin.
            # four separate tiles so the two rings never co-write one tile
            # (Tile serializes cross-queue writers of a shared tile)
            t16_0 = []
            for k in range(KT):
                t0k = tin.tile([128, H], F16, tag="t0k", name=f"t0k_{k}")
                eng = nc.sync if k < 2 else nc.scalar
                eng.dma_start(t0k[:], t16d[0, 128 * k : 128 * (k + 1), :])
                t16_0.append(t0k)
            t16s.append(t16_0)
            # b1 also arrives per-k (single ring, one tile) so its first
            # k-tiles are resident before b0's block finishes
            t16_1 = tin.tile([128, KT, H], F16, tag="t16")
            for k in range(KT):
                nc.sync.dma_start(
                    t16_1[:, k, :], t16d[1, 128 * k : 128 * (k + 1), :]
                )
            t16s.append(t16_1)
            for b in range(2, BS):
                t16 = tin.tile([128, KT, H], F16, tag="t16")
                nc.sync.dma_start(
                    t16[:], t16d[b].rearrange("(k p) h -> p k h", p=128)
                )
                t16s.append(t16)
            for b in range(1, BS):
                at = ain_pool.tile([128, KT, EM], F16, tag="at")
                nc.scalar.dma_start(at[:], a_in[b])
                ats.append(at)
            gel_sb = consts.tile([128, 16], F32R)
            nc.scalar.dma_start(gel_sb[:], gel[:])
            aent3_sb = consts.tile([128, KT, E], F16)
            nc.scalar.dma_start(aent3_sb[:], aent3[:])

            # PE warm-up: HAM releases the clock throttle after ~3.4us of
            # sustained matmul activity; burn that in during the DMA head so
            # the real matmuls run at 2.4 GHz from the start.
            scratch = consts.tile([128, 512], F16)
            nc.gpsimd.memset(scratch[:], 0.0)
            wm = psq.tile([128, 512], F32, tag="ps")
            for _ in range(12):
                nc.tensor.matmul(
                    wm[:], lhsT=scratch[:, 0:128], rhs=scratch[:],
                    start=True, stop=True,
                )

            def mention_block(b):
                # mentions = (1/S) * A^T @ t   (two 128-row chunks of em)
                mfs = []
                for e2 in range(2):
                    pmh = [
                        psq.tile([128, 512], F32, tag="ps", name=f"pm_{b}_{e2}_{h}")
                        for h in range(2)
                    ]
                    for k in range(KT):
                        for h2 in range(2):
                            nc.tensor.matmul(
                                pmh[h2][:, 0:HH],
                                lhsT=ats[b][:, k, 128 * e2 : 128 * (e2 + 1)],
                                rhs=(
                                    t16s[b][k][:, HH * h2 : HH * (h2 + 1)]
                                    if b == 0
                                    else t16s[b][:, k, HH * h2 : HH * (h2 + 1)]
                                ),
                                start=(k == 0),
                                stop=(k == KT - 1),
                            )
                    mf = mf32p.tile([128, 2, HH], F32R, tag="mf")
                    for h2 in range(2):
                        nc.scalar.activation(
                            mf[:, h2, :], pmh[h2][:, 0:HH], ACTF.Copy, 0.0, 1.0 / S
                        )
                    nc.sync.dma_start(
                        mentions[b, 128 * e2 : 128 * (e2 + 1), :].rearrange(
                            "p (c h) -> p c h", c=2
                        ),
                        mf[:].bitcast(F32),
                    )
                    mfs.append(mf)
                return mfs

            def entity_block(b, mfs):
                # entity = Gel^T @ mentions  (float32r full-rate matmul)
                for e2 in range(2):
                    peh = [
                        psq.tile([16, 512], F32, tag="ps", name=f"pe_{b}_{e2}_{h}")
                        for h in range(2)
                    ]
                    ent = entp.tile([16, 2, HH], F32, tag="ent")
                    for h2 in range(2):
                        nc.tensor.matmul(
                            peh[h2][:, 0:HH],
                            lhsT=gel_sb[:],
                            rhs=mfs[e2][:, h2, :],
                            start=True,
                            stop=True,
                        )
                        nc.vector.tensor_scalar_mul(
                            ent[:, h2, :], peh[h2][:, 0:HH], 1.0
                        )
                    nc.sync.dma_start(
                        entity[b, 16 * e2 : 16 * (e2 + 1), :].rearrange(
                            "p (g h) -> p g h", g=2
                        ),
                        ent[:],
                    )

            def entity_block_direct(b):
                # entity = (1/(S*M)) * Aent^T @ t — no dependency on the
                # mention copies, so the kernel tail isn't gated on ACT
                peh3 = [
                    psq.tile([32, 512], F32, tag="ps", name=f"pe3_{h}")
                    for h in range(2)
                ]
                ent = entp.tile([32, 2, HH], F32, tag="ent")
                for h2 in range(2):
                    for k in range(KT):
                        nc.tensor.matmul(
                            peh3[h2][:, 0:HH],
                            lhsT=aent3_sb[:, k, :],
                            rhs=t16s[b][:, k, HH * h2 : HH * (h2 + 1)],
                            start=(k == 0),
                            stop=(k == KT - 1),
                        )
                    nc.vector.tensor_scalar_mul(
                        ent[:, h2, :], peh3[h2][:, 0:HH], 1.0 / (S * M)
                    )
                nc.sync.dma_start(
                    entity[b].rearrange("e (g h) -> e g h", g=2),
                    ent[:],
                )

            # entity block for example b runs one mention-block late so its
            # mf dependency (an ACT copy) is long satisfied when PE gets
            # there; the direct (token-contracting) block for the last
            # example runs even earlier so only short copies trail the
            # final mention block
            all_mfs = []
            for b in range(BS):
                if b == BS - 1:
                    entity_block_direct(BS - 1)
                all_mfs.append(mention_block(b))
                if b >= 1:
                    entity_block(b - 1, all_mfs[b - 1])
    nc.compile()
    return nc


_NC = None


def _get_nc():
    global _NC
    if _NC is None:
        _NC = build_nc()
    return _NC


def make_host_inputs(token_reprs, pos_idx):
    """Split + preprocess full inputs into per-core in_maps."""
    token_reprs = np.asarray(token_reprs)
    pos = np.asarray(pos_idx).astype(np.int64)
    assert token_reprs.shape == (B, L, H)
    assert pos.shape == (B, E, M, S)
    t16 = np.ascontiguousarray(token_reprs.astype(np.float16))

    # dense one-hot count encoding of pos_idx: A[b, l, em] = #{s: pos==l}
    pos_flat = pos.reshape(B, EM, S)
    b_idx = np.arange(B)[:, None, None]
    em_idx = np.arange(EM)[None, :, None]
    lin = (b_idx * L + pos_flat) * EM + em_idx
    counts = np.bincount(lin.ravel(), minlength=B * L * EM).reshape(B, L, EM)
    # a_in[b, p, k, em]
    a_np = np.ascontiguousarray(
        counts.reshape(B, KT, 128, EM).transpose(0, 2, 1, 3).astype(np.float16)
    )
    # aent[b, p, k, e] = sum_m counts
    aent_np = np.ascontiguousarray(
        counts.reshape(B, KT, 128, E, M).sum(axis=4).transpose(0, 2, 1, 3)
        .astype(np.float16)
    )
    gel = np.zeros((128, 16), np.float32)
    gel[np.arange(128), np.arange(128) // M] = 1.0 / M

    in_maps = []
    for c in range(N_CORES):
        sl = slice(c * BS, (c + 1) * BS)
        in_maps.append(
            {
                "t16": t16[sl],
                "a_in": a_np[sl],
                "aent3": aent_np[c * BS + BS - 1],
                "gel": gel,
            }
        )
    return in_maps


def kernel(token_reprs, pos_idx):
    global LAST_RESULTS
    in_maps = make_host_inputs(token_reprs, pos_idx)
    nc = _get_nc()
    res = run_bass_kernel_spmd(nc, in_maps, list(range(N_CORES)), trace=TRACE)
    LAST_RESULTS = res
    mentions = np.concatenate([r["mentions"] for r in res.results], axis=0).reshape(
        B, E, M, H
    )
    entity = np.concatenate([r["entity"] for r in res.results], axis=0)
    mask = np.ones((B, E, M), np.float32)
    return entity, mentions, mask


# revision 50
# speedup vs baseline: 1.1454x; 1.0472x over previous
"""Trainium2 Bass kernel for nn_EntityRepr (segment_reduce).

Computation (per example):
  gathered[e,m,s,:] = token_reprs[pos[e,m,s], :]
  mentions[e,m,:]   = mean_s gathered
  entity[e,:]       = mean_m mentions
  mask              = ones

Strategy: data-parallel over batch (4 examples per core, 8 cores).
Per example, the gather+mean runs as matmuls on the PE with one-hot
count matrices as the stationary operand:
  A_k[p, em] = #{s : pos[em,s] == 128k+p}     (dense encoding of pos_idx,
                                               prepared host-side)
  mentions   = (1/S) * A^T @ t                (PE, fp16, K=L=512)
  entity     = Gel^T @ mentions               (PE, float32r, from the f32
                                               mention tiles in SBUF)
The entity block for example b is emitted one mention-block late so its
ACT-copy dependency is long satisfied when the in-order PE reaches it;
the last example's entity instead contracts directly against the tokens
with Aent = sum_m A (host-prepared), so the kernel tail never waits on
a PSUM->SBUF copy.

DMA queues: inputs + entity outputs ride the Sync HWDGE ring, mention
outputs the Scalar HWDGE ring (separate FIFOs).
"""

import os
import numpy as np

import concourse.bass as bass
import concourse.bacc as bacc
import concourse.mybir as mybir
from concourse.bass_utils import run_bass_kernel_spmd
from concourse.tile import TileContext

N_CORES = 8
B, L, H = 32, 512, 768
E, M, S = 32, 8, 4
BS = B // N_CORES          # examples per core
EM = E * M                 # 256 mentions / example
EMS = EM * S               # 1024 gathered rows / example
KT = L // 128              # 4 contraction tiles
HH = H // 2                # 384, matmul free-dim tile

F16 = mybir.dt.float16
F32 = mybir.dt.float32
F32R = mybir.dt.float32r
ALU = mybir.AluOpType
ACTF = mybir.ActivationFunctionType

# set by test harness to collect a profile
TRACE = bool(int(os.environ.get("KERNEL_TRACE", "0")))
LAST_RESULTS = None


def build_nc():
    nc = bacc.Bacc(None, target_bir_lowering=False)
    t16d = nc.declare_dram_parameter("t16", [BS, L, H], F16, isOutput=False)
    # a_in[b, p, k, em] = #{s : pos[b, em, s] == 128k+p}
    a_in = nc.declare_dram_parameter("a_in", [BS, 128, KT, EM], F16, isOutput=False)
    # aent3[p, k, e] = sum_m a_in[BS-1, p, k, e*M+m]
    aent3 = nc.declare_dram_parameter("aent3", [128, KT, E], F16, isOutput=False)
    gel = nc.declare_dram_parameter("gel", [128, 16], F32R, isOutput=False)
    mentions = nc.declare_dram_parameter("mentions", [BS, EM, H], F32, isOutput=True)
    entity = nc.declare_dram_parameter("entity", [BS, E, H], F32, isOutput=True)

    with TileContext(nc) as tc:
        with (
            tc.tile_pool(name="consts", bufs=1) as consts,
            tc.tile_pool(name="tin", bufs=4) as tin,
            tc.tile_pool(name="ain", bufs=4) as ain_pool,
            tc.tile_pool(name="mf32p", bufs=8) as mf32p,
            tc.tile_pool(name="entp", bufs=4) as entp,
            tc.tile_pool(name="psq", bufs=8, space="PSUM") as psq,
        ):
            # input DMAs, earliest-needed first; one-hot counts ride the
            # Scalar ring, tokens the Sync ring, so the two first-example
            # dependencies land in parallel. b0's tokens arrive per-k so the
            # first matmul can start as soon as k=0 is resident.
            ats, t16s = [], []
            at0 = ain_pool.tile([128, KT, EM], F16, tag="at")
            nc.scalar.dma_start(at0[:], a_in[0])
            ats.append(at0)
            # b0's tokens split per-k across both rings so the first mention
            # block can start as soon as k=0 lands
            # b0's k0/k1 on sync, k2/k3 on scalar: both rings are otherwise
            # idle during the head, so all four k-tiles land ~in parallel.
            # The b1..b3 conveyor follows immediately on sync, ahead of any
            # output traffic; the scalar ring stays clear once ACT copies
            # begin.
            # four separate tiles so the two rings never co-write one tile
            # (Tile serializes cross-queue writers of a shared tile)
            t16_0 = []
            for k in range(KT):
                t0k = tin.tile([128, H], F16, tag="t0k", name=f"t0k_{k}")
                eng = nc.sync if k < 2 else nc.scalar
                eng.dma_start(t0k[:], t16d[0, 128 * k : 128 * (k + 1), :])
                t16_0.append(t0k)
            t16s.append(t16_0)
            # b1 also arrives per-k (single ring, one tile) so its first
            # k-tiles are resident before b0's block finishes
            t16_1 = tin.tile([128, KT, H], F16, tag="t16")
            for k in range(KT):
                nc.sync.dma_start(
                    t16_1[:, k, :], t16d[1, 128 * k : 128 * (k + 1), :]
                )
            t16s.append(t16_1)
            for b in range(2, BS):
                t16 = tin.tile([128, KT, H], F16, tag="t16")
                nc.sync.dma_start(
                    t16[:], t16d[b].rearrange("(k p) h -> p k h", p=128)
                )
                t16s.append(t16)
            for b in range(1, BS):
                at = ain_pool.tile([128, KT, EM], F16, tag="at")
                nc.scalar.dma_start(at[:], a_in[b])
                ats.append(at)
            gel_sb = consts.tile([128, 16], F32R)
            nc.scalar.dma_start(gel_sb[:], gel[:])
            aent3_sb = consts.tile([128, KT, E], F16)
            nc.scalar.dma_start(aent3_sb[:], aent3[:])

            # PE warm-up: HAM releases the clock throttle after ~3.4us of
            # sustained matmul activity; burn that in during the DMA head so
            # the real matmuls run at 2.4 GHz from the start.
            scratch = consts.tile([128, 512], F16)
            nc.gpsimd.memset(scratch[:], 0.0)
            wm = psq.tile([128, 512], F32, tag="ps")
            for _ in range(11):
                nc.tensor.matmul(
                    wm[:], lhsT=scratch[:, 0:128], rhs=scratch[:],
                    start=True, stop=True,
                )

            def mention_block(b):
                # mentions = (1/S) * A^T @ t   (two 128-row chunks of em)
                mfs = []
                for e2 in range(2):
                    pmh = [
                        psq.tile([128, 512], F32, tag="ps", name=f"pm_{b}_{e2}_{h}")
                        for h in range(2)
                    ]
                    for k in range(KT):
                        for h2 in range(2):
                            nc.tensor.matmul(
                                pmh[h2][:, 0:HH],
                                lhsT=ats[b][:, k, 128 * e2 : 128 * (e2 + 1)],
                                rhs=(
                                    t16s[b][k][:, HH * h2 : HH * (h2 + 1)]
                                    if b == 0
                                    else t16s[b][:, k, HH * h2 : HH * (h2 + 1)]
                                ),
                                start=(k == 0),
                                stop=(k == KT - 1),
                            )
                    mf = mf32p.tile([128, 2, HH], F32R, tag="mf")
                    for h2 in range(2):
                        nc.scalar.activation(
                            mf[:, h2, :], pmh[h2][:, 0:HH], ACTF.Copy, 0.0, 1.0 / S
                        )
                    nc.sync.dma_start(
                        mentions[b, 128 * e2 : 128 * (e2 + 1), :].rearrange(
                            "p (c h) -> p c h", c=2
                        ),
                        mf[:].bitcast(F32),
                    )
                    mfs.append(mf)
                return mfs

            def entity_block(b, mfs):
                # entity = Gel^T @ mentions  (float32r full-rate matmul)
                for e2 in range(2):
                    peh = [
                        psq.tile([16, 512], F32, tag="ps", name=f"pe_{b}_{e2}_{h}")
                        for h in range(2)
                    ]
                    ent = entp.tile([16, 2, HH], F32, tag="ent")
                    for h2 in range(2):
                        nc.tensor.matmul(
                            peh[h2][:, 0:HH],
                            lhsT=gel_sb[:],
                            rhs=mfs[e2][:, h2, :],
                            start=True,
                            stop=True,
                        )
                        nc.vector.tensor_scalar_mul(
                            ent[:, h2, :], peh[h2][:, 0:HH], 1.0
                        )
                    nc.sync.dma_start(
                        entity[b, 16 * e2 : 16 * (e2 + 1), :].rearrange(
                            "p (g h) -> p g h", g=2
                        ),
                        ent[:],
                    )

            def entity_block_direct(b):
                # entity = (1/(S*M)) * Aent^T @ t — no dependency on the
                # mention copies, so the kernel tail isn't gated on ACT
                peh3 = [
                    psq.tile([32, 512], F32, tag="ps", name=f"pe3_{h}")
                    for h in range(2)
                ]
                ent = entp.tile([32, 2, HH], F32, tag="ent")
                for h2 in range(2):
                    for k in range(KT):
                        nc.tensor.matmul(
                            peh3[h2][:, 0:HH],
                            lhsT=aent3_sb[:, k, :],
                            rhs=t16s[b][:, k, HH * h2 : HH * (h2 + 1)],
                            start=(k == 0),
                            stop=(k == KT - 1),
                        )
                    nc.vector.tensor_scalar_mul(
                        ent[:, h2, :], peh3[h2][:, 0:HH], 1.0 / (S * M)
                    )
                nc.sync.dma_start(
                    entity[b].rearrange("e (g h) -> e g h", g=2),
                    ent[:],
                )

            # entity block for example b runs one mention-block late so its
            # mf dependency (an ACT copy) is long satisfied when PE gets
            # there; the direct (token-contracting) block for the last
            # example runs even earlier so only short copies trail the
            # final mention block
            all_mfs = []
            for b in range(BS):
                if b == BS - 1:
                    entity_block_direct(BS - 1)
                all_mfs.append(mention_block(b))
                if b >= 1:
                    entity_block(b - 1, all_mfs[b - 1])
    nc.compile()
    return nc


_NC = None


def _get_nc():
    global _NC
    if _NC is None:
        _NC = build_nc()
    return _NC


def make_host_inputs(token_reprs, pos_idx):
    """Split + preprocess full inputs into per-core in_maps."""
    token_reprs = np.asarray(token_reprs)
    pos = np.asarray(pos_idx).astype(np.int64)
    assert token_reprs.shape == (B, L, H)
    assert pos.shape == (B, E, M, S)
    t16 = np.ascontiguousarray(token_reprs.astype(np.float16))

    # dense one-hot count encoding of pos_idx: A[b, l, em] = #{s: pos==l}
    pos_flat = pos.reshape(B, EM, S)
    b_idx = np.arange(B)[:, None, None]
    em_idx = np.arange(EM)[None, :, None]
    lin = (b_idx * L + pos_flat) * EM + em_idx
    counts = np.bincount(lin.ravel(), minlength=B * L * EM).reshape(B, L, EM)
    # a_in[b, p, k, em]
    a_np = np.ascontiguousarray(
        counts.reshape(B, KT, 128, EM).transpose(0, 2, 1, 3).astype(np.float16)
    )
    # aent[b, p, k, e] = sum_m counts
    aent_np = np.ascontiguousarray(
        counts.reshape(B, KT, 128, E, M).sum(axis=4).transpose(0, 2, 1, 3)
        .astype(np.float16)
    )
    gel = np.zeros((128, 16), np.float32)
    gel[np.arange(128), np.arange(128) // M] = 1.0 / M

    in_maps = []
    for c in range(N_CORES):
        sl = slice(c * BS, (c + 1) * BS)
        in_maps.append(
            {
                "t16": t16[sl],
                "a_in": a_np[sl],
                "aent3": aent_np[c * BS + BS - 1],
                "gel": gel,
            }
        )
    return in_maps


def kernel(token_reprs, pos_idx):
    global LAST_RESULTS
    in_maps = make_host_inputs(token_reprs, pos_idx)
    nc = _get_nc()
    res = run_bass_kernel_spmd(nc, in_maps, list(range(N_CORES)), trace=TRACE)
    LAST_RESULTS = res
    mentions = np.concatenate([r["mentions"] for r in res.results], axis=0).reshape(
        B, E, M, H
    )
    entity = np.concatenate([r["entity"] for r in res.results], axis=0)
    mask = np.ones((B, E, M), np.float32)
    return entity, mentions, mask
